# revision 2
# baseline (speedup 1.0000x reference)
"""MoE top-1 routing layer on 8 Trainium2 NeuronCores (expert-parallel).

Math: out[t] = (x[t] @ W[e] + b[e]) @ OW + ob   with e = argmax(x[t] @ GW + gb).

Decomposition used here:
  out[t] = (x[t] @ W[e]) @ OW + bias2[e],   bias2[e] = b[e] @ OW + ob
so the device only runs two chained matmuls per core; the per-expert bias
constant is added by the host during unshard.

Sharding: expert-parallel. Host computes the gate (fp64 -> exact argmax),
sorts tokens by expert, pads each expert's token set to capacity C, and
ships core k: xT (gathered tokens, transposed), W[k], OW. Each core returns
its C token outputs; host scatters rows back and adds bias2. Tokens beyond
capacity (never for balanced routing) fall back to a host matmul.
"""

import numpy as np
from contextlib import ExitStack

B, S, D, E, H, O = 4, 2048, 1024, 8, 2048, 1024
T = B * S
C = 1152          # per-expert token capacity (multiple of 128)
P = 128
KO_D = D // P     # 8
KO_H = H // P     # 16
BLK = 384         # token block (<=512 so one fp32 matmul per c-block)
N_BLK = C // BLK  # 3
HH = H // 2       # h-half for streamed W


def _legalize_waits(nc):
    """This container's walrus accepts 1 sem wait per instruction (2 for
    EventSemaphore); Tile's tail drain can carry more. Split the excess
    onto preceding same-engine NoOps."""
    from concourse import mybir

    uid = 0
    for f in nc.m.functions:
        for b in f.blocks:
            insts = b.instructions
            out = []
            changed = False
            for ins in insts:
                si = ins.sync_info
                waits = list(si.on_wait) if si is not None else []
                limit = 2 if str(ins.opcode) == "EventSemaphore" else 1
                if len(waits) > limit:
                    extra, keep = waits[:-limit], waits[-limit:]
                    for w in extra:
                        uid += 1
                        out.append(
                            mybir.InstNoOp(
                                name=f"waitsplit-{uid}",
                                engine=ins.engine,
                                sync_info=mybir.SyncInfo(on_wait=[w], on_update=[]),
                                bass_nofuse=True,
                            )
                        )
                    si.on_wait = keep
                    changed = True
                out.append(ins)
            if changed:
                insts.clear()
                insts.extend(out)


def _build_nc():
    import concourse.bass as bass
    import concourse.tile as tile
    from concourse import mybir

    f32 = mybir.dt.float32
    nc = bass.Bass()
    xt = nc.dram_tensor("xt", [D, C], f32, kind="ExternalInput")
    w = nc.dram_tensor("w", [D, H], f32, kind="ExternalInput")
    ow = nc.dram_tensor("ow", [H, O], f32, kind="ExternalInput")
    out = nc.dram_tensor("out", [C, O], f32, kind="ExternalOutput")

    xt_r = xt.rearrange("(ko p) c -> p ko c", p=P)    # [128, 8, C]
    w_r = w.rearrange("(ko p) h -> p ko h", p=P)      # [128, 8, H]
    ow_r = ow.rearrange("(ko p) o -> p ko o", p=P)    # [128, 16, O]

    with tile.TileContext(nc) as tc:
        with ExitStack() as ctx:
            ow_pool = ctx.enter_context(tc.tile_pool(name="ow", bufs=1))
            x_pool = ctx.enter_context(tc.tile_pool(name="x", bufs=2))
            w_pool = ctx.enter_context(tc.tile_pool(name="w", bufs=2))
            h1_pool = ctx.enter_context(tc.tile_pool(name="h1", bufs=1))
            st_pool = ctx.enter_context(tc.tile_pool(name="st", bufs=3))
            ps_pool = ctx.enter_context(
                tc.tile_pool(name="ps", bufs=4, space="PSUM")
            )

            # out-proj weights resident: [128, 16, O] (64 KB/partition)
            ow_sb = ow_pool.tile([P, KO_H, O], f32)
            for k in range(KO_H):
                nc.sync.dma_start(ow_sb[:, k], ow_r[:, k])

            for blk in range(N_BLK):
                c0 = blk * BLK
                # token block x: [128, 8, BLK]
                x_sb = x_pool.tile([P, KO_D, BLK], f32)
                nc.sync.dma_start(x_sb[:], xt_r[:, :, c0 : c0 + BLK])

                h1_sb = h1_pool.tile([P, KO_H, BLK], f32)

                # ---- mm1: h1[h, c] = sum_d W[d, h] * x[d, c] ----
                for half in range(2):
                    # W h-half stripes: [128, 8, HH] (contiguous 4KB rows)
                    w_sb = w_pool.tile([P, KO_D, HH], f32)
                    for k in range(KO_D):
                        nc.sync.dma_start(
                            w_sb[:, k], w_r[:, k, half * HH : (half + 1) * HH]
                        )
                    for hi in range(HH // P):
                        h = half * (HH // P) + hi
                        ps = ps_pool.tile([P, 512], f32, name="ps1")[:, :BLK]
                        for k in range(KO_D):
                            nc.tensor.matmul(
                                ps,
                                w_sb[:, k, hi * P : (hi + 1) * P],
                                x_sb[:, k],
                                start=(k == 0),
                                stop=(k == KO_D - 1),
                            )
                        nc.vector.tensor_copy(h1_sb[:, h], ps)

                # ---- mm2: out[c, o] = sum_h h1[h, c] * OW[h, o] ----
                for t in range(BLK // P):
                    st = st_pool.tile([P, O], f32)
                    for o2 in range(O // 512):
                        ps2 = ps_pool.tile([P, 512], f32)
                        for kh in range(KO_H):
                            nc.tensor.matmul(
                                ps2,
                                h1_sb[:, kh, t * P : (t + 1) * P],
                                ow_sb[:, kh, o2 * 512 : (o2 + 1) * 512],
                                start=(kh == 0),
                                stop=(kh == KO_H - 1),
                            )
                        nc.vector.tensor_copy(st[:, o2 * 512 : (o2 + 1) * 512], ps2)
                    r0 = c0 + t * P
                    nc.sync.dma_start(out[r0 : r0 + P, :], st[:])

    _legalize_waits(nc)
    return nc


_NC_CACHE = {}


def kernel(x, gate_w, gate_b, expert_w, expert_b, out_w, out_b):
    from concourse.bass_utils import run_bass_kernel_spmd

    x = np.asarray(x, dtype=np.float32)
    gate_w = np.asarray(gate_w, dtype=np.float32)
    gate_b = np.asarray(gate_b, dtype=np.float32)
    expert_w = np.asarray(expert_w, dtype=np.float32)
    expert_b = np.asarray(expert_b, dtype=np.float32)
    out_w = np.asarray(out_w, dtype=np.float32)
    out_b = np.asarray(out_b, dtype=np.float32)

    xt = x.reshape(T, D)
    # Gate on host in fp64: argmax then exact match with the fp32 reference
    # (min top-2 logit gap is ~1e-5, fp64 error ~1e-12).
    logits = xt.astype(np.float64) @ gate_w.astype(np.float64) + gate_b.astype(
        np.float64
    )
    idx = np.argmax(logits, axis=1)

    tok_of_expert = [np.nonzero(idx == e)[0] for e in range(E)]
    in_maps = []
    kept = []
    overflow = []
    for e in range(E):
        toks = tok_of_expert[e]
        if len(toks) > C:
            overflow.append((e, toks[C:]))
            toks = toks[:C]
        kept.append(toks)
        xT_e = np.zeros((D, C), dtype=np.float32)
        xT_e[:, : len(toks)] = xt[toks].T
        in_maps.append(
            {
                "xt": xT_e,
                "w": np.ascontiguousarray(expert_w[e]),
                "ow": out_w,
            }
        )

    if "nc" not in _NC_CACHE:
        _NC_CACHE["nc"] = _build_nc()
    nc = _NC_CACHE["nc"]

    res = run_bass_kernel_spmd(nc, in_maps, list(range(E)))

    bias2 = (
        expert_b.astype(np.float64) @ out_w.astype(np.float64)
        + out_b.astype(np.float64)
    ).astype(np.float32)  # [E, O]

    out = np.empty((T, O), dtype=np.float32)
    for e in range(E):
        toks = kept[e]
        out[toks] = res.results[e]["out"][: len(toks)] + bias2[e]
    for e, toks in overflow:
        h1 = xt[toks] @ expert_w[e]
        out[toks] = h1 @ out_w + bias2[e]
    return out.reshape(B, S, O)


# revision 3
# speedup vs baseline: 2.8587x; 2.8587x over previous
"""MoE top-1 routing layer on 8 Trainium2 NeuronCores (expert-parallel).

Math: out[t] = (x[t] @ W[e] + b[e]) @ OW + ob   with e = argmax(x[t] @ GW + gb).

Decomposition used here:
  out[t] = (x[t] @ W[e]) @ OW + bias2[e],   bias2[e] = b[e] @ OW + ob
so the device only runs two chained matmuls per core; the per-expert bias
constant is added by the host during unshard.

Sharding: expert-parallel. Host computes the gate (fp64 -> exact argmax),
sorts tokens by expert, pads each expert's token set to capacity C, and
ships core k: xT (gathered tokens, transposed), W[k], OW. Each core returns
its C token outputs; host scatters rows back and adds bias2. Tokens beyond
capacity (never for balanced routing) fall back to a host matmul.
"""

import numpy as np
from contextlib import ExitStack

B, S, D, E, H, O = 4, 2048, 1024, 8, 2048, 1024
T = B * S
C = 1152          # per-expert token capacity (multiple of 128)
P = 128
KO_D = D // P     # 8
KO_H = H // P     # 16
BLK = 384         # token block (<=512 so one fp32 matmul per c-block)
N_BLK = C // BLK  # 3
HH = H // 2       # h-half for streamed W


def _legalize_waits(nc):
    """This container's walrus accepts 1 sem wait per instruction (2 for
    EventSemaphore); Tile's tail drain can carry more. Split the excess
    onto preceding same-engine NoOps."""
    from concourse import mybir

    uid = 0
    for f in nc.m.functions:
        for b in f.blocks:
            insts = b.instructions
            out = []
            changed = False
            for ins in insts:
                si = ins.sync_info
                waits = list(si.on_wait) if si is not None else []
                limit = 2 if str(ins.opcode) == "EventSemaphore" else 1
                if len(waits) > limit:
                    extra, keep = waits[:-limit], waits[-limit:]
                    for w in extra:
                        uid += 1
                        out.append(
                            mybir.InstNoOp(
                                name=f"waitsplit-{uid}",
                                engine=ins.engine,
                                sync_info=mybir.SyncInfo(on_wait=[w], on_update=[]),
                                bass_nofuse=True,
                            )
                        )
                    si.on_wait = keep
                    changed = True
                out.append(ins)
            if changed:
                insts.clear()
                insts.extend(out)


def _build_nc():
    import concourse.bass as bass
    import concourse.tile as tile
    from concourse import mybir

    f32 = mybir.dt.float32
    f32r = mybir.dt.float32r  # fp32 storage; PE streams 1 col/cycle (vs 4 for fp32)
    nc = bass.Bass()
    xt = nc.dram_tensor("xt", [D, C], f32r, kind="ExternalInput")
    w = nc.dram_tensor("w", [D, H], f32r, kind="ExternalInput")
    ow = nc.dram_tensor("ow", [H, O], f32r, kind="ExternalInput")
    out = nc.dram_tensor("out", [C, O], f32, kind="ExternalOutput")

    xt_r = xt.rearrange("(ko p) c -> p ko c", p=P)    # [128, 8, C]
    w_r = w.rearrange("(ko p) h -> p ko h", p=P)      # [128, 8, H]
    ow_r = ow.rearrange("(ko p) o -> p ko o", p=P)    # [128, 16, O]

    with tile.TileContext(nc) as tc:
        with ExitStack() as ctx:
            ow_pool = ctx.enter_context(tc.tile_pool(name="ow", bufs=1))
            x_pool = ctx.enter_context(tc.tile_pool(name="x", bufs=2))
            w_pool = ctx.enter_context(tc.tile_pool(name="w", bufs=2))
            h1_pool = ctx.enter_context(tc.tile_pool(name="h1", bufs=1))
            st_pool = ctx.enter_context(tc.tile_pool(name="st", bufs=3))
            ps_pool = ctx.enter_context(
                tc.tile_pool(name="ps", bufs=4, space="PSUM")
            )

            # out-proj weights resident: [128, 16, O] (64 KB/partition)
            ow_sb = ow_pool.tile([P, KO_H, O], f32r)
            for k in range(KO_H):
                nc.sync.dma_start(ow_sb[:, k], ow_r[:, k])

            for blk in range(N_BLK):
                c0 = blk * BLK
                # token block x: [128, 8, BLK]
                x_sb = x_pool.tile([P, KO_D, BLK], f32r)
                nc.sync.dma_start(x_sb[:], xt_r[:, :, c0 : c0 + BLK])

                h1_sb = h1_pool.tile([P, KO_H, BLK], f32r)

                # ---- mm1: h1[h, c] = sum_d W[d, h] * x[d, c] ----
                for half in range(2):
                    # W h-half stripes: [128, 8, HH] (contiguous 4KB rows)
                    w_sb = w_pool.tile([P, KO_D, HH], f32r)
                    for k in range(KO_D):
                        nc.sync.dma_start(
                            w_sb[:, k], w_r[:, k, half * HH : (half + 1) * HH]
                        )
                    for hi in range(HH // P):
                        h = half * (HH // P) + hi
                        ps = ps_pool.tile([P, 512], f32, name="ps1")[:, :BLK]
                        for k in range(KO_D):
                            nc.tensor.matmul(
                                ps,
                                w_sb[:, k, hi * P : (hi + 1) * P],
                                x_sb[:, k],
                                start=(k == 0),
                                stop=(k == KO_D - 1),
                            )
                        nc.vector.tensor_copy(h1_sb[:, h], ps)

                # ---- mm2: out[c, o] = sum_h h1[h, c] * OW[h, o] ----
                for t in range(BLK // P):
                    st = st_pool.tile([P, O], f32)
                    for o2 in range(O // 512):
                        ps2 = ps_pool.tile([P, 512], f32)
                        for kh in range(KO_H):
                            nc.tensor.matmul(
                                ps2,
                                h1_sb[:, kh, t * P : (t + 1) * P],
                                ow_sb[:, kh, o2 * 512 : (o2 + 1) * 512],
                                start=(kh == 0),
                                stop=(kh == KO_H - 1),
                            )
                        nc.vector.tensor_copy(st[:, o2 * 512 : (o2 + 1) * 512], ps2)
                    r0 = c0 + t * P
                    nc.sync.dma_start(out[r0 : r0 + P, :], st[:])

    _legalize_waits(nc)
    return nc


_NC_CACHE = {}


def kernel(x, gate_w, gate_b, expert_w, expert_b, out_w, out_b):
    from concourse.bass_utils import run_bass_kernel_spmd

    x = np.asarray(x, dtype=np.float32)
    gate_w = np.asarray(gate_w, dtype=np.float32)
    gate_b = np.asarray(gate_b, dtype=np.float32)
    expert_w = np.asarray(expert_w, dtype=np.float32)
    expert_b = np.asarray(expert_b, dtype=np.float32)
    out_w = np.asarray(out_w, dtype=np.float32)
    out_b = np.asarray(out_b, dtype=np.float32)

    xt = x.reshape(T, D)
    # Gate on host in fp64: argmax then exact match with the fp32 reference
    # (min top-2 logit gap is ~1e-5, fp64 error ~1e-12).
    logits = xt.astype(np.float64) @ gate_w.astype(np.float64) + gate_b.astype(
        np.float64
    )
    idx = np.argmax(logits, axis=1)

    tok_of_expert = [np.nonzero(idx == e)[0] for e in range(E)]
    in_maps = []
    kept = []
    overflow = []
    for e in range(E):
        toks = tok_of_expert[e]
        if len(toks) > C:
            overflow.append((e, toks[C:]))
            toks = toks[:C]
        kept.append(toks)
        xT_e = np.zeros((D, C), dtype=np.float32)
        xT_e[:, : len(toks)] = xt[toks].T
        in_maps.append(
            {
                "xt": xT_e,
                "w": np.ascontiguousarray(expert_w[e]),
                "ow": out_w,
            }
        )

    if "nc" not in _NC_CACHE:
        _NC_CACHE["nc"] = _build_nc()
    nc = _NC_CACHE["nc"]

    res = run_bass_kernel_spmd(nc, in_maps, list(range(E)))

    bias2 = (
        expert_b.astype(np.float64) @ out_w.astype(np.float64)
        + out_b.astype(np.float64)
    ).astype(np.float32)  # [E, O]

    out = np.empty((T, O), dtype=np.float32)
    for e in range(E):
        toks = kept[e]
        out[toks] = res.results[e]["out"][: len(toks)] + bias2[e]
    for e, toks in overflow:
        h1 = xt[toks] @ expert_w[e]
        out[toks] = h1 @ out_w + bias2[e]
    return out.reshape(B, S, O)


# revision 5
# speedup vs baseline: 3.6186x; 1.2658x over previous
"""MoE top-1 routing layer on 8 Trainium2 NeuronCores (expert-parallel).

Math: out[t] = (x[t] @ W[e] + b[e]) @ OW + ob   with e = argmax(x[t] @ GW + gb).

Decomposition used here:
  out[t] = (x[t] @ W[e]) @ OW + bias2[e],   bias2[e] = b[e] @ OW + ob
so the device only runs two chained matmuls per core; the per-expert bias
constant is added by the host during unshard.

Sharding: expert-parallel. Host computes the gate (fp64 -> exact argmax),
sorts tokens by expert, pads each expert's token set to capacity C, and
ships core k: xT (gathered tokens, transposed), W[k], OW. Each core returns
its C token outputs; host scatters rows back and adds bias2. Tokens beyond
capacity (never for balanced routing) fall back to a host matmul.
"""

import numpy as np
from contextlib import ExitStack

B, S, D, E, H, O = 4, 2048, 1024, 8, 2048, 1024
T = B * S
C = 1152          # per-expert token capacity (multiple of 128)
P = 128
KO_D = D // P     # 8
KO_H = H // P     # 16
BLK = 384         # token block (<=512 so one fp32 matmul per c-block)
N_BLK = C // BLK  # 3
HH = H // 2       # h-half for streamed W


def _legalize_waits(nc):
    """This container's walrus accepts 1 sem wait per instruction (2 for
    EventSemaphore); Tile's tail drain can carry more. Split the excess
    onto preceding same-engine NoOps."""
    from concourse import mybir

    uid = 0
    for f in nc.m.functions:
        for b in f.blocks:
            insts = b.instructions
            out = []
            changed = False
            for ins in insts:
                si = ins.sync_info
                waits = list(si.on_wait) if si is not None else []
                limit = 2 if str(ins.opcode) == "EventSemaphore" else 1
                if len(waits) > limit:
                    extra, keep = waits[:-limit], waits[-limit:]
                    for w in extra:
                        uid += 1
                        out.append(
                            mybir.InstNoOp(
                                name=f"waitsplit-{uid}",
                                engine=ins.engine,
                                sync_info=mybir.SyncInfo(on_wait=[w], on_update=[]),
                                bass_nofuse=True,
                            )
                        )
                    si.on_wait = keep
                    changed = True
                out.append(ins)
            if changed:
                insts.clear()
                insts.extend(out)


# "bf16": all matmul operands bf16 (fp32 PSUM accumulation) — fastest DMA/PE,
#         rel err ~2e-3. "f32r": fp32-width storage, reduced-precision multiply,
#         rel err ~2e-4, more DMA (weights re-streamed per token block).
MM_DT = "bf16"


def _build_nc_bf16():
    """Single pass over C tokens; x, W, OW all SBUF-resident in bf16."""
    import concourse.bass as bass
    import concourse.tile as tile
    from concourse import mybir

    f32 = mybir.dt.float32
    bf16 = mybir.dt.bfloat16
    CS = 384  # mm1 token sub-tile (N=384 keeps PE ahead of FWL ldweights)

    nc = bass.Bass()
    xt = nc.dram_tensor("xt", [D, C], bf16, kind="ExternalInput")
    w = nc.dram_tensor("w", [D, H], bf16, kind="ExternalInput")
    ow = nc.dram_tensor("ow", [H, O], bf16, kind="ExternalInput")
    out = nc.dram_tensor("out", [C, O], f32, kind="ExternalOutput")

    xt_r = xt.rearrange("(ko p) c -> p ko c", p=P)    # [128, 8, C]
    w_r = w.rearrange("(ko p) h -> p ko h", p=P)      # [128, 8, H]
    ow_r = ow.rearrange("(ko p) o -> p ko o", p=P)    # [128, 16, O]

    with tile.TileContext(nc) as tc:
        with ExitStack() as ctx:
            x_pool = ctx.enter_context(tc.tile_pool(name="x", bufs=1))
            w_pool = ctx.enter_context(tc.tile_pool(name="w", bufs=1))
            ow_pool = ctx.enter_context(tc.tile_pool(name="ow", bufs=1))
            h1_pool = ctx.enter_context(tc.tile_pool(name="h1", bufs=1))
            st_pool = ctx.enter_context(tc.tile_pool(name="st", bufs=3))
            ps_pool = ctx.enter_context(
                tc.tile_pool(name="ps", bufs=4, space="PSUM")
            )

            # Everything resident; DMA issue order = demand order.
            x_sb = x_pool.tile([P, KO_D, C], bf16)       # 18 KB/part
            w_sb = w_pool.tile([P, KO_D, H], bf16)       # 32 KB/part
            ow_sb = ow_pool.tile([P, KO_H, O], bf16)     # 32 KB/part
            h1_sb = h1_pool.tile([P, KO_H, C], bf16)     # 36 KB/part

            for k in range(KO_D):
                nc.sync.dma_start(x_sb[:, k], xt_r[:, k])
            # W in h-quarters so the first mm1 group is gated on ~1 MB
            for q in range(4):
                for k in range(KO_D):
                    nc.sync.dma_start(
                        w_sb[:, k, q * 512 : (q + 1) * 512],
                        w_r[:, k, q * 512 : (q + 1) * 512],
                    )
            for k in range(KO_H):
                nc.sync.dma_start(ow_sb[:, k], ow_r[:, k])

            # ---- mm1: h1[h, c] = sum_d W[d, h] * x[d, c] ----
            for h in range(KO_H):
                for cs in range(C // CS):
                    ps = ps_pool.tile([P, CS], f32, name="ps1")
                    for k in range(KO_D):
                        nc.tensor.matmul(
                            ps,
                            w_sb[:, k, h * P : (h + 1) * P],
                            x_sb[:, k, cs * CS : (cs + 1) * CS],
                            start=(k == 0),
                            stop=(k == KO_D - 1),
                        )
                    nc.vector.tensor_copy(h1_sb[:, h, cs * CS : (cs + 1) * CS], ps)

            # ---- mm2: out[c, o] = sum_h h1[h, c] * OW[h, o] ----
            for t in range(C // P):
                st = st_pool.tile([P, O], f32)
                for o2 in range(O // 512):
                    ps2 = ps_pool.tile([P, 512], f32, name="ps2")
                    for kh in range(KO_H):
                        nc.tensor.matmul(
                            ps2,
                            h1_sb[:, kh, t * P : (t + 1) * P],
                            ow_sb[:, kh, o2 * 512 : (o2 + 1) * 512],
                            start=(kh == 0),
                            stop=(kh == KO_H - 1),
                        )
                    nc.vector.tensor_copy(st[:, o2 * 512 : (o2 + 1) * 512], ps2)
                nc.sync.dma_start(out[t * P : (t + 1) * P, :], st[:])

    _legalize_waits(nc)
    return nc


def _build_nc():
    if MM_DT == "bf16":
        return _build_nc_bf16()
    import concourse.bass as bass
    import concourse.tile as tile
    from concourse import mybir

    f32 = mybir.dt.float32
    f32r = mybir.dt.float32r  # fp32 storage; PE streams 1 col/cycle (vs 4 for fp32)
    nc = bass.Bass()
    xt = nc.dram_tensor("xt", [D, C], f32r, kind="ExternalInput")
    w = nc.dram_tensor("w", [D, H], f32r, kind="ExternalInput")
    ow = nc.dram_tensor("ow", [H, O], f32r, kind="ExternalInput")
    out = nc.dram_tensor("out", [C, O], f32, kind="ExternalOutput")

    xt_r = xt.rearrange("(ko p) c -> p ko c", p=P)    # [128, 8, C]
    w_r = w.rearrange("(ko p) h -> p ko h", p=P)      # [128, 8, H]
    ow_r = ow.rearrange("(ko p) o -> p ko o", p=P)    # [128, 16, O]

    with tile.TileContext(nc) as tc:
        with ExitStack() as ctx:
            ow_pool = ctx.enter_context(tc.tile_pool(name="ow", bufs=1))
            x_pool = ctx.enter_context(tc.tile_pool(name="x", bufs=2))
            w_pool = ctx.enter_context(tc.tile_pool(name="w", bufs=2))
            h1_pool = ctx.enter_context(tc.tile_pool(name="h1", bufs=1))
            st_pool = ctx.enter_context(tc.tile_pool(name="st", bufs=3))
            ps_pool = ctx.enter_context(
                tc.tile_pool(name="ps", bufs=4, space="PSUM")
            )

            # out-proj weights resident: [128, 16, O] (64 KB/partition)
            ow_sb = ow_pool.tile([P, KO_H, O], f32r)
            for k in range(KO_H):
                nc.sync.dma_start(ow_sb[:, k], ow_r[:, k])

            for blk in range(N_BLK):
                c0 = blk * BLK
                # token block x: [128, 8, BLK]
                x_sb = x_pool.tile([P, KO_D, BLK], f32r)
                nc.sync.dma_start(x_sb[:], xt_r[:, :, c0 : c0 + BLK])

                h1_sb = h1_pool.tile([P, KO_H, BLK], f32r)

                # ---- mm1: h1[h, c] = sum_d W[d, h] * x[d, c] ----
                for half in range(2):
                    # W h-half stripes: [128, 8, HH] (contiguous 4KB rows)
                    w_sb = w_pool.tile([P, KO_D, HH], f32r)
                    for k in range(KO_D):
                        nc.sync.dma_start(
                            w_sb[:, k], w_r[:, k, half * HH : (half + 1) * HH]
                        )
                    for hi in range(HH // P):
                        h = half * (HH // P) + hi
                        ps = ps_pool.tile([P, 512], f32, name="ps1")[:, :BLK]
                        for k in range(KO_D):
                            nc.tensor.matmul(
                                ps,
                                w_sb[:, k, hi * P : (hi + 1) * P],
                                x_sb[:, k],
                                start=(k == 0),
                                stop=(k == KO_D - 1),
                            )
                        nc.vector.tensor_copy(h1_sb[:, h], ps)

                # ---- mm2: out[c, o] = sum_h h1[h, c] * OW[h, o] ----
                for t in range(BLK // P):
                    st = st_pool.tile([P, O], f32)
                    for o2 in range(O // 512):
                        ps2 = ps_pool.tile([P, 512], f32)
                        for kh in range(KO_H):
                            nc.tensor.matmul(
                                ps2,
                                h1_sb[:, kh, t * P : (t + 1) * P],
                                ow_sb[:, kh, o2 * 512 : (o2 + 1) * 512],
                                start=(kh == 0),
                                stop=(kh == KO_H - 1),
                            )
                        nc.vector.tensor_copy(st[:, o2 * 512 : (o2 + 1) * 512], ps2)
                    r0 = c0 + t * P
                    nc.sync.dma_start(out[r0 : r0 + P, :], st[:])

    _legalize_waits(nc)
    return nc


_NC_CACHE = {}


def kernel(x, gate_w, gate_b, expert_w, expert_b, out_w, out_b):
    from concourse.bass_utils import run_bass_kernel_spmd

    x = np.asarray(x, dtype=np.float32)
    gate_w = np.asarray(gate_w, dtype=np.float32)
    gate_b = np.asarray(gate_b, dtype=np.float32)
    expert_w = np.asarray(expert_w, dtype=np.float32)
    expert_b = np.asarray(expert_b, dtype=np.float32)
    out_w = np.asarray(out_w, dtype=np.float32)
    out_b = np.asarray(out_b, dtype=np.float32)

    xt = x.reshape(T, D)
    # Gate on host in fp64: argmax then exact match with the fp32 reference
    # (min top-2 logit gap is ~1e-5, fp64 error ~1e-12).
    logits = xt.astype(np.float64) @ gate_w.astype(np.float64) + gate_b.astype(
        np.float64
    )
    idx = np.argmax(logits, axis=1)

    if MM_DT == "bf16":
        import ml_dtypes

        mm_np = ml_dtypes.bfloat16
    else:
        mm_np = np.float32

    tok_of_expert = [np.nonzero(idx == e)[0] for e in range(E)]
    in_maps = []
    kept = []
    overflow = []
    ow_dev = np.ascontiguousarray(out_w.astype(mm_np))
    for e in range(E):
        toks = tok_of_expert[e]
        if len(toks) > C:
            overflow.append((e, toks[C:]))
            toks = toks[:C]
        kept.append(toks)
        xT_e = np.zeros((D, C), dtype=mm_np)
        xT_e[:, : len(toks)] = xt[toks].T.astype(mm_np)
        in_maps.append(
            {
                "xt": xT_e,
                "w": np.ascontiguousarray(expert_w[e].astype(mm_np)),
                "ow": ow_dev,
            }
        )

    if "nc" not in _NC_CACHE:
        _NC_CACHE["nc"] = _build_nc()
    nc = _NC_CACHE["nc"]

    res = run_bass_kernel_spmd(nc, in_maps, list(range(E)))

    bias2 = (
        expert_b.astype(np.float64) @ out_w.astype(np.float64)
        + out_b.astype(np.float64)
    ).astype(np.float32)  # [E, O]

    out = np.empty((T, O), dtype=np.float32)
    for e in range(E):
        toks = kept[e]
        out[toks] = res.results[e]["out"][: len(toks)] + bias2[e]
    for e, toks in overflow:
        h1 = xt[toks] @ expert_w[e]
        out[toks] = h1 @ out_w + bias2[e]
    return out.reshape(B, S, O)
